# revision 1
# baseline (speedup 1.0000x reference)
"""Single-head causal attention (B=8, T=2048, C=256, H=64) on 8 TRN2 NeuronCores.

Sharding: batch dim across the 8 cores (data parallel, one batch element per
core); each core computes its full TxT causal attention independently.

Per-core algorithm (x_b = x[b], shape [T, C]), all matmul operands bf16:
  proj_a = [Wq|Wk].T @ x_b.T   [128, T]  (q rows 0:64, k rows 64:128)
  proj_b = [Wk|Wq].T @ x_b.T   [128, T]  (j<=1 via matmul, j>1 via SBUF swap)
  v[tk, h] per 128-chunk: xt_chunk.T @ Wv  -> vaug [tk, 65] with ones col
  per tq block j (512 wide), per tk chunk pair: dual row-group QK into PSUM
  (diagonal pairs packed compactly so exp touches no dead columns), exp on
  ACT -> bf16 e in SBUF, triangular mask on diagonal 128-blocks (DVE 4x),
  PV accumulates outT_aug[h+1, 512] in PSUM; epilogue transposes via PE,
  per-row reciprocal normalize, DMA out (partition-major; host un-permutes).

Emission is software-pipelined: QK/exp run one pair ahead of PV, each
epilogue sinks two pairs into the next block, and proj(j+1) is hoisted into
block j so no engine head-of-line blocks on a cross-engine dependency.
"""

import numpy as np

import concourse.bass as bass
import concourse.mybir as mybir
import concourse.tile as tile
from concourse import bass_utils

B, T, C, H = 8, 2048, 256, 64
NCC = C // 128          # 2 c-chunks
NTQ = T // 512          # 4 tq blocks
NTK = T // 128          # 16 tk chunks

dt = mybir.dt
BF = dt.bfloat16
F32 = dt.float32
U16 = dt.uint16

# Schraudolph bf16 exp constants: exp(x) ~= bitcast_bf16(uint16(A16*x + B16))
# (DVE rounds on float->uint16 convert; max rel err ~3.3%, washed out by the
# softmax normalization). The 0.125 attention scale is folded into EXP_A.
EXP_A = 0.125 * (1 << 7) / float(np.log(2))
EXP_B = 127.0 * (1 << 7) - 5.5

# number of full (off-diagonal) pairs per tq block whose exp runs on the DVE
# via the bit-trick instead of the ACT engine (load balancing; 0 everywhere —
# the PE is the pole, and DVE congestion delays masks/PVs more than the ACT
# relief is worth)
DVE_EXP = (0, 0, 0, 0)

# wmisc packed column offsets (all bf16): wa cc0|cc1, wb cc0|cc1, wv cc0|cc1,
# tri, idn
WA_OFF = 0
WB_OFF = 256
WV_OFF = 512
TRI_OFF = 640
IDN_OFF = 768
WMISC_COLS = 896

LABELS = {}                 # instruction name -> human label (debug/trace aid)


def L(bi, label):
    try:
        LABELS[bi.ins.name] = label
    except Exception:
        pass
    return bi


def _split_excess_waits(nc, max_waits=1):
    """The walrus build in this container rejects >1 sync wait per
    instruction ("Too many sync wait commands"); spill extras onto
    preceding same-engine NoOps (same AND semantics, engine blocks at the
    NoOp until the semaphore condition holds)."""
    for f in nc.m.functions:
        for bb in f.blocks:
            new = []
            for inst in bb.instructions:
                si = inst.sync_info
                waits = list(si.on_wait) if si is not None else []
                if len(waits) > max_waits:
                    extra, keep = waits[:-max_waits], waits[-max_waits:]
                    for i in range(0, len(extra), max_waits):
                        chunk = extra[i:i + max_waits]
                        nop = mybir.InstNoOp(
                            name=nc.get_next_instruction_name(),
                            engine=inst.engine,
                            ins=[], outs=[],
                            sync_info=mybir.SyncInfo(on_wait=chunk, on_update=[]),
                        )
                        nc.register_instruction(nop)
                        new.append(nop)
                    inst.sync_info = mybir.SyncInfo(
                        on_wait=keep, on_update=list(si.on_update))
                new.append(inst)
            bb.instructions = new


def _patch_tile_drain():
    """Tile's kernel-tail drain carries one wait per live semaphore; split
    them the same way (idempotent monkeypatch)."""
    from concourse.vector_clock import ScopedClock

    if getattr(tile.TileContext, "_ant_drain_patched", False):
        return

    def _drain_and_barrier(self, tick_clock, wait_clock):
        drain_inst = self.nc.sync.drain()
        wait_clock.add_sem_waits(
            drain_inst.ins, ScopedClock({None: tick_clock.global_clock}))
        si = drain_inst.ins.sync_info
        waits = list(si.on_wait) if si is not None else []
        if len(waits) > 1:
            drain_inst.ins.sync_info = mybir.SyncInfo(
                on_wait=[waits[0]], on_update=list(si.on_update))
            for w in waits[1:]:
                ni = self.nc.sync.nop(nofuse=True)
                ni.ins.sync_info = mybir.SyncInfo(on_wait=[w], on_update=[])
        self.nc.all_engine_barrier()
        assert self.sems is not None
        popped = self.nc._tile_sem_poison_stack.pop()
        assert popped is self._sem_poison
        self.nc.clear_and_free_semaphores(list(self.sems.allocated().values()))
        self.nc.all_engine_barrier()

    tile.TileContext._drain_and_barrier = _drain_and_barrier
    tile.TileContext._ant_drain_patched = True


def _attention_body(nc, tc, pools, dram, prio=True, max_units=None):
    """Emit one pass of the per-core attention computation."""
    persist, epool, onat, psw, pso, pmisc = pools
    xt_d, wm_d, out_d = dram
    Exp = mybir.ActivationFunctionType.Exp

    # ---- persistent SBUF tensors -------------------------------------
    xt = persist.tile([128, NCC, T], BF, tag="xt")
    wm = persist.tile([128, WMISC_COLS], BF, tag="wm")
    proj_a = persist.tile([128, T], BF, tag="proj_a")   # [qT; kT]
    proj_b = persist.tile([128, T], BF, tag="proj_b")   # [kT; qT]
    # inner dim padded to H+2 so each chunk slab is 4-byte aligned
    vaug = persist.tile([128, NTK, H + 2], BF, tag="vaug")

    tri = wm[:, TRI_OFF:TRI_OFF + 128]
    idn = wm[:, IDN_OFF:IDN_OFF + 128]

    # ones column of vaug (rowsum trick); no other writer touches col 64
    nc.gpsimd.memset(vaug[:, :, H:H + 1], 1.0)

    # preload the ACT exp table immediately (scratch input, no DMA dep)
    warm = onat.tile([1, 16], F32, tag="warm")
    nc.vector.memset(warm[:], 0.0)
    nc.scalar.activation(warm[:], warm[:], Exp, scale=1.0)

    # ---- input DMAs: ordered by first use so nothing head-of-line blocks --
    L(nc.sync.dma_start(wm[:, 0:WV_OFF], wm_d[:, 0:WV_OFF]), "dma_wm0")
    L(nc.sync.dma_start(xt[:, 0, 0:512], xt_d[:, 0, 0:512]), "dma_xt0c0")
    L(nc.sync.dma_start(xt[:, 1, 0:512], xt_d[:, 1, 0:512]), "dma_xt0c1")
    L(nc.sync.dma_start(xt[:, :, 512:1024], xt_d[:, :, 512:1024]), "dma_xt1")
    L(nc.sync.dma_start(xt[:, :, 1024:1536], xt_d[:, :, 1024:1536]), "dma_xt2")
    L(nc.sync.dma_start(wm[:, WV_OFF:], wm_d[:, WV_OFF:]), "dma_wm1")
    L(nc.sync.dma_start(xt[:, :, 1536:2048], xt_d[:, :, 1536:2048]), "dma_xt3")

    po_tiles = {}

    def emit_proj(j):
        sl = slice(512 * j, 512 * (j + 1))
        pp = pmisc.tile([128, 512], F32, tag="pm")
        for cc in range(NCC):
            L(nc.tensor.matmul(pp[:], wm[:, WA_OFF + 128 * cc:WA_OFF + 128 * (cc + 1)],
                             xt[:, cc, sl], start=(cc == 0), stop=(cc == NCC - 1)),
              f"mm_pp{j}c{cc}")
        L(nc.vector.tensor_copy(proj_a[:, sl], pp[:]), f"cp_pa{j}")
        # matmul-built swap (a proj_b SBUF-to-SBUF DMA round-trip loses more
        # to DMA latency/scheduling than the duplicate matmuls cost the PE)
        pb = pmisc.tile([128, 512], F32, tag="pm")
        for cc in range(NCC):
            L(nc.tensor.matmul(pb[:], wm[:, WB_OFF + 128 * cc:WB_OFF + 128 * (cc + 1)],
                             xt[:, cc, sl], start=(cc == 0), stop=(cc == NCC - 1)),
              f"mm_pb{j}c{cc}")
        if j == 0:
            L(nc.scalar.copy(proj_b[:, sl], pb[:]), f"cp_pb{j}")   # ACT idle pre-exp
        else:
            L(nc.vector.tensor_copy(proj_b[:, sl], pb[:]), f"cp_pb{j}")

    def emit_vproj(j):
        # v chunks directly in [tk, h] orientation (no transpose needed)
        pv = pmisc.tile([128, 512], F32, tag="pm")
        for t in range(4):
            for cc in range(NCC):
                L(nc.tensor.matmul(pv[:, 64 * t:64 * (t + 1)],
                                 xt[:, cc, 512 * j + 128 * t:512 * j + 128 * (t + 1)],
                                 wm[:, WV_OFF + 64 * cc:WV_OFF + 64 * (cc + 1)],
                                 start=(cc == 0), stop=(cc == NCC - 1)), f"mm_v{j}t{t}c{cc}")
        L(nc.vector.tensor_copy(vaug[:, 4 * j:4 * j + 4, 0:H], pv[:, 0:256]), f"cp_v{j}")

    def make_pair(j, i0, i1, first, last, use_dve=False):
        """Return (qk_exp_mask_fn, pv_fn) closures for chunk pair (i0, i1).

        Chunk i0's strip sits at wp cols [0:s0] (PSUM bank 0), chunk i1's at
        [512:512+s1] (bank 1) — the two dual row-group matmuls must land in
        different PSUM banks (same-bank dual groups fault on hardware).
        Diagonal chunks are paired (0,3)/(1,2) so strip widths fill banks
        with at most 128 dead columns under the single spanning exp.
        """
        d0, d1 = i0 - 4 * j, i1 - 4 * j
        c0 = 128 * max(0, d0)
        c1 = 128 * max(0, d1)
        s1 = 512 - c1
        state = {}

        def qk_exp_mask():
            wp = psw.tile([128, 1024], F32, tag="w")
            # strip0 at [c0:512] in bank 0, strip1 at [512:512+s1] in bank 1;
            # the single exp spans [c0 : 512+s1], all of it written by the
            # two matmuls (no dead columns)
            L(nc.tensor.matmul(wp[:, c0:512],
                             proj_b[0:64, 128 * i0:128 * (i0 + 1)],
                             proj_a[0:64, 512 * j + c0:512 * (j + 1)],
                             start=True, stop=True), f"QK_j{j}i{i0}")
            L(nc.tensor.matmul(wp[:, 512:512 + s1],
                             proj_a[64:128, 128 * i1:128 * (i1 + 1)],
                             proj_b[64:128, 512 * j + c1:512 * (j + 1)],
                             start=True, stop=True, tile_position=(64, 0)), f"QK_j{j}i{i1}")
            if use_dve:
                e = epool.tile([128, 1024], U16, tag="e", name=f"eu{j}_{i0}")
                with tc.high_priority(offset=-5000):
                    L(nc.vector.tensor_scalar(
                        e[:, c0:512 + s1], wp[:, c0:512 + s1], EXP_A, EXP_B,
                        mybir.AluOpType.mult, mybir.AluOpType.add),
                      f"dexp_j{j}i{i0}.{i1}")
            else:
                e = epool.tile([128, 1024], BF, tag="e", name=f"e{j}_{i0}")
                L(nc.scalar.activation(e[:, c0:512 + s1], wp[:, c0:512 + s1], Exp,
                                     scale=0.125), f"exp_j{j}i{i0}.{i1}")
            state["e"] = e

        def pv():
            e = state.pop("e")
            if d0 >= 0:
                # triangular mask on each chunk's diagonal 128-block
                L(nc.vector.tensor_mul(e[:, c0:c0 + 128], e[:, c0:c0 + 128], tri), f"mask_j{j}i{i0}")
                L(nc.vector.tensor_mul(e[:, 512:640], e[:, 512:640], tri), f"mask_j{j}i{i1}")
            if use_dve:
                m0 = e[:, c0:512].bitcast(BF)
                m1 = e[:, 512:512 + s1].bitcast(BF)
            else:
                m0 = e[:, c0:512]
                m1 = e[:, 512:512 + s1]
            po = po_tiles[j]
            L(nc.tensor.matmul(po[:, c0:512], vaug[:, i0, 0:H + 1], m0,
                             start=first, stop=False,
                             skip_group_check=True), f"PV_j{j}i{i0}")
            L(nc.tensor.matmul(po[:, c1:512], vaug[:, i1, 0:H + 1], m1,
                             start=False, stop=last,
                             skip_group_check=True), f"PV_j{j}i{i1}")

        return qk_exp_mask, pv

    def emit_epi(j):
        sl_out = slice(4 * j, 4 * (j + 1))
        po = po_tiles.pop(j)
        ot = onat.tile([H + 1, 512], BF, tag="ot")
        if j == NTQ - 1:
            L(nc.scalar.copy(ot[:], po[:]), f"cp_ot{j}")   # ACT idle after its last exp
        else:
            L(nc.vector.tensor_copy(ot[:], po[:]), f"cp_ot{j}")
        # inner dim padded to 66 so each [*, t, :] slab is 4-byte aligned
        # (PSUM writes require 4-byte alignment; 65 bf16 = 130 bytes)
        pt = pmisc.tile([128, 4, H + 2], BF, tag="pm")
        for t in range(4):
            L(nc.tensor.transpose(pt[:, t, 0:H + 1], ot[:, 128 * t:128 * (t + 1)],
                                idn[0:H + 1, 0:H + 1]), f"tp{j}t{t}")
        rc = onat.tile([128, 4, 1], F32, tag="rc")
        L(nc.vector.reciprocal(rc[:], pt[:, :, H:H + 1]), f"rc{j}")
        on = onat.tile([128, 4, H], F32, tag="on")
        L(nc.vector.tensor_mul(on[:], pt[:, :, 0:H],
                             rc[:].to_broadcast([128, 4, H])), f"on{j}")
        L(nc.sync.dma_start(out_d[:, sl_out, :], on[:]), f"dma_out{j}")

    # ---- build the global pair stream (diagonal pairs first per block) ---
    stream = []                 # (j, pair_index_within_block, qk_fn, pv_fn)
    for j in range(NTQ):
        nk = 4 * j + 4
        order = [(4 * j, 4 * j + 3), (4 * j + 1, 4 * j + 2)]
        order += [(2 * p, 2 * p + 1) for p in range(2 * j)]
        for idx, (i0, i1) in enumerate(order):
            qk, pv = make_pair(j, i0, i1, first=(idx == 0),
                               last=(idx == len(order) - 1),
                               use_dve=(idx >= 2 and
                                        idx >= len(order) - DVE_EXP[j]))
            stream.append((j, idx, qk, pv))

    # ---- software-pipelined emission -------------------------------------
    # The Tile scheduler is a dep-driven priority list scheduler (emission
    # order = priority tiebreak). Mark the QK->exp chain high priority so a
    # ready QK always preempts filler work on the PE; keep PV/proj/epilogue
    # at normal priority as fillers.
    from contextlib import nullcontext

    if prio:
        hp = tc.high_priority                       # raise: emission prio 0
        lp = lambda off: tc.high_priority(offset=-off)   # demote by off
    else:
        hp = nullcontext
        lp = lambda off: nullcontext()

    emit_proj(0)
    if max_units is not None:
        stream = stream[:max_units]
    pv_pending = []
    for j, idx, qk, pv in stream:
        if idx == 0:
            po_tiles[j] = pso.tile([H + 1, 512], F32, tag="o", name=f"po{j}")
        with hp():
            qk()
        if j == 0 and idx == 0:
            with hp():
                emit_proj(1)
        if j == 0 and idx == 1:
            with hp():
                emit_vproj(0)
                emit_vproj(1)
                emit_proj(2)
        if j == 1 and idx == 1:
            with hp():
                emit_proj(3)
                emit_vproj(2)
        if j == 2 and idx == 1:
            with hp():
                emit_vproj(3)
        if idx == 0 and j >= 1:
            # epilogue of the previous block must precede this block's first
            # PV (pso slot reuse is write-after-read against its po read),
            # which is popped one unit later at idx == 1
            with lp(10000):
                pv_pending.pop(0)()
            with lp(12000):
                emit_epi(j - 1)
        elif pv_pending:
            with lp(10000):
                pv_pending.pop(0)()
        pv_pending.append(pv)
    for pv in pv_pending:
        with hp():
            pv()
    if max_units is None:
        with hp():
            emit_epi(NTQ - 1)


def build_nc(repeats=1, mm_dt=None):
    """Build the per-core Bass program (SPMD: same program on all 8 cores).

    repeats > 1 wraps the body in an on-device For_i loop; used only by the
    benchmarking harness to amortize host/launch overhead out of timing.
    """
    _patch_tile_drain()
    nc = bass.Bass("TRN2", target_bir_lowering=False, debug=False)

    xt_d = nc.dram_tensor("xt", [128, NCC, T], BF, kind="ExternalInput")
    wm_d = nc.dram_tensor("wm", [128, WMISC_COLS], BF, kind="ExternalInput")
    # partition-major output: out[p, t, h] = attention(row 128*t + p)
    out_d = nc.dram_tensor("out", [128, NTK, H], F32, kind="ExternalOutput")
    dram = (xt_d, wm_d, out_d)

    with tile.TileContext(nc) as tc:
        with (
            tc.tile_pool(name="persist", bufs=1) as persist,
            tc.tile_pool(name="epool", bufs=14) as epool,
            tc.tile_pool(name="onat", bufs=4) as onat,
            tc.tile_pool(name="psw", bufs=2, space="PSUM") as psw,
            tc.tile_pool(name="pso", bufs=1, space="PSUM") as pso,
            tc.tile_pool(name="pmisc", bufs=3, space="PSUM") as pmisc,
        ):
            pools = (persist, epool, onat, psw, pso, pmisc)
            if repeats == 1:
                _attention_body(nc, tc, pools, dram)
            else:
                with tc.For_i(0, repeats, 1):
                    _attention_body(nc, tc, pools, dram)
    _split_excess_waits(nc)
    return nc


def make_in_maps(x, Wk, Wq, Wv):
    """Host-side layout prep: per-core transposed x, packed weights, masks."""
    import ml_dtypes
    bf16 = ml_dtypes.bfloat16

    x = np.asarray(x, dtype=np.float32)
    Wk = np.asarray(Wk, dtype=np.float32)
    Wq = np.asarray(Wq, dtype=np.float32)
    Wv = np.asarray(Wv, dtype=np.float32)

    wa = np.concatenate([Wq, Wk], axis=1).reshape(NCC, 128, 128)
    wb = np.concatenate([Wk, Wq], axis=1).reshape(NCC, 128, 128)
    wv = Wv.reshape(NCC, 128, H)
    r = np.arange(128)
    tri = (r[:, None] <= r[None, :]).astype(np.float32)  # keep tk <= tq
    idn = np.eye(128, dtype=np.float32)
    wm = np.concatenate(
        [wa[0], wa[1], wb[0], wb[1], wv[0], wv[1], tri, idn],
        axis=1).astype(bf16)
    assert wm.shape == (128, WMISC_COLS)

    common = {"wm": np.ascontiguousarray(wm)}
    in_maps = []
    for b in range(B):
        # x[b].T is [C, T] with c = 128*cc + p -> [p, cc, t]
        xt = np.ascontiguousarray(
            np.transpose(x[b].T.reshape(NCC, 128, T), (1, 0, 2))).astype(bf16)
        in_maps.append({"xt": xt, **common})
    return in_maps


def unpack_out(raw):
    """Device layout [128, NTK, H] (partition-major) -> [T, H]."""
    return np.ascontiguousarray(
        np.transpose(raw, (1, 0, 2)).reshape(T, H))


def kernel(x, Wk, Wq, Wv):
    nc = build_nc(repeats=1)
    in_maps = make_in_maps(x, Wk, Wq, Wv)
    res = bass_utils.run_bass_kernel_spmd(nc, in_maps, core_ids=list(range(B)))
    return np.stack([unpack_out(res.results[b]["out"]) for b in range(B)],
                    axis=0)



# revision 2
# speedup vs baseline: 4.2769x; 4.2769x over previous
"""Single-head causal attention (B=8, T=2048, C=256, H=64) on 8 TRN2 NeuronCores.

Sharding: batch dim across the 8 cores (data parallel, one batch element per
core); each core computes its full TxT causal attention independently.

Same per-core algorithm as kernel2 (transposed PV), but the benchmark loop
is software-pipelined across iterations: projections for iteration i+1 are
computed as low-priority PE/DVE/ACT filler while iteration i's attention
runs, with ping-pong proj/vaug/xt buffers (For_i body holds two phases).
Steady state therefore has no serial projection head: each phase's first QK
fires as soon as the PE drains the previous phase.

Per-phase attention (proj_a = [q;k], proj_b = [k;q] precomputed):
  per tq block j (512 wide), per tk chunk pair: dual row-group QK into PSUM
  (diagonal pairs packed so exp touches no dead columns), exp on ACT (or
  DVE Schraudolph bit-trick for balance) -> bf16 e, triangular mask on
  diagonal 128-blocks (DVE), transposed PV (stationary = e 128-col slice,
  moving = vaug) accumulating po[tq, H+1] in PSUM, per-block normalize by
  the ones-column rowsum, out DMA from the Pool DGE queue.
"""

import numpy as np

import concourse.bass as bass
import concourse.mybir as mybir
import concourse.tile as tile
from concourse import bass_utils

B, T, C, H = 8, 2048, 256, 64
NCC = C // 128          # 2 c-chunks
NTQ = T // 512          # 4 tq blocks
NTK = T // 128          # 16 tk chunks

dt = mybir.dt
BF = dt.bfloat16
F32 = dt.float32
U16 = dt.uint16

EXP_A = 0.125 * (1 << 7) / float(np.log(2))
EXP_B = 127.0 * (1 << 7) - 5.5

DVE_EXP = (0, 0, 2, 2)
DVE_EXP_PLACE = "spread"
PSW_BUFS = 2
PSO_BUFS = 2
PMISC_BUFS = 2
MASK_ENG = "vector"
CPV_ENG = "vector"

# wmisc packed column offsets (all bf16): wa cc0|cc1, wv cc0|cc1, tri, perm
WA_OFF = 0
WV_OFF = 256
TRI_OFF = 384
PERM_OFF = 512
WMISC_COLS = 640

LABELS = {}


def L(bi, label):
    try:
        LABELS[bi.ins.name] = label
    except Exception:
        pass
    return bi


def _dve_units(j, n_units):
    n = DVE_EXP[j]
    offs = list(range(2, n_units))
    if not n or not offs:
        return set()
    if DVE_EXP_PLACE == "early":
        return set(offs[:n])
    if DVE_EXP_PLACE == "late":
        return set(offs[-n:])
    step = max(1, len(offs) // n)
    return set(offs[::step][:n])


def _split_excess_waits(nc, max_waits=1):
    """The walrus build in this container rejects >1 sync wait per
    instruction; spill extras onto preceding same-engine NoOps."""
    for f in nc.m.functions:
        for bb in f.blocks:
            new = []
            for inst in bb.instructions:
                si = inst.sync_info
                waits = list(si.on_wait) if si is not None else []
                if len(waits) > max_waits:
                    extra, keep = waits[:-max_waits], waits[-max_waits:]
                    for i in range(0, len(extra), max_waits):
                        chunk = extra[i:i + max_waits]
                        nop = mybir.InstNoOp(
                            name=nc.get_next_instruction_name(),
                            engine=inst.engine,
                            ins=[], outs=[],
                            sync_info=mybir.SyncInfo(on_wait=chunk, on_update=[]),
                        )
                        nc.register_instruction(nop)
                        new.append(nop)
                    inst.sync_info = mybir.SyncInfo(
                        on_wait=keep, on_update=list(si.on_update))
                new.append(inst)
            bb.instructions = new


def _patch_tile_drain():
    """Split the multi-wait kernel-tail drain the same way (idempotent)."""
    from concourse.vector_clock import ScopedClock

    if getattr(tile.TileContext, "_ant_drain_patched", False):
        return

    def _drain_and_barrier(self, tick_clock, wait_clock):
        drain_inst = self.nc.sync.drain()
        wait_clock.add_sem_waits(
            drain_inst.ins, ScopedClock({None: tick_clock.global_clock}))
        si = drain_inst.ins.sync_info
        waits = list(si.on_wait) if si is not None else []
        if len(waits) > 1:
            drain_inst.ins.sync_info = mybir.SyncInfo(
                on_wait=[waits[0]], on_update=list(si.on_update))
            for w in waits[1:]:
                ni = self.nc.sync.nop(nofuse=True)
                ni.ins.sync_info = mybir.SyncInfo(on_wait=[w], on_update=[])
        self.nc.all_engine_barrier()
        assert self.sems is not None
        popped = self.nc._tile_sem_poison_stack.pop()
        assert popped is self._sem_poison
        self.nc.clear_and_free_semaphores(list(self.sems.allocated().values()))
        self.nc.all_engine_barrier()

    tile.TileContext._drain_and_barrier = _drain_and_barrier
    tile.TileContext._ant_drain_patched = True


class Ctx:
    """Shared emission context."""

    def __init__(self, nc, tc, pools, wm, projs, dram):
        self.nc, self.tc = nc, tc
        (self.xtp, self.epool, self.onat,
         self.psw, self.pso, self.pmisc) = pools
        self.wm = wm
        self.projs = projs      # [(proj_a, proj_b, vaug, xt), ...] x2
        self.xt_d, self.wm_d, self.out_d = dram


def emit_xt_dma(cx, ph):
    nc = cx.nc
    xt = cx.projs[ph][3]
    L(nc.sync.dma_start(xt[:, 0, 0:512], cx.xt_d[:, 0, 0:512]), "dma_xt0c0")
    L(nc.sync.dma_start(xt[:, 1, 0:512], cx.xt_d[:, 1, 0:512]), "dma_xt0c1")
    L(nc.sync.dma_start(xt[:, :, 512:1024], cx.xt_d[:, :, 512:1024]), "dma_xt1")
    L(nc.sync.dma_start(xt[:, :, 1024:1536], cx.xt_d[:, :, 1024:1536]), "dma_xt2")
    L(nc.sync.dma_start(xt[:, :, 1536:2048], cx.xt_d[:, :, 1536:2048]), "dma_xt3")


def emit_proj(cx, ph, j):
    """Projection block j into proj set ph (reads xt[ph])."""
    nc = cx.nc
    proj_a, proj_b, vaug, xt = cx.projs[ph]
    wm = cx.wm
    perm = wm[:, PERM_OFF:PERM_OFF + 128]
    sl = slice(512 * j, 512 * (j + 1))
    pp = cx.pmisc.tile([128, 512], F32, tag="pm")
    for cc in range(NCC):
        L(nc.tensor.matmul(pp[:], wm[:, WA_OFF + 128 * cc:WA_OFF + 128 * (cc + 1)],
                         xt[:, cc, sl], start=(cc == 0), stop=(cc == NCC - 1)),
          f"mm_pp{j}")
    L(nc.vector.tensor_copy(proj_a[:, sl], pp[:]), f"cp_pa{j}")
    # proj_b = row-halves swap of proj_a via a PE permutation matmul
    pb = cx.pmisc.tile([128, 512], F32, tag="pm")
    L(nc.tensor.matmul(pb[:], perm, proj_a[:, sl], start=True, stop=True),
      f"mm_pb{j}")
    L(nc.scalar.copy(proj_b[:, sl], pb[:]), f"cp_pb{j}")


def emit_vproj(cx, ph, j):
    nc = cx.nc
    proj_a, proj_b, vaug, xt = cx.projs[ph]
    wm = cx.wm
    pv = cx.pmisc.tile([128, 512], F32, tag="pm")
    for t in range(4):
        for cc in range(NCC):
            L(nc.tensor.matmul(pv[:, 64 * t:64 * (t + 1)],
                             xt[:, cc, 512 * j + 128 * t:512 * j + 128 * (t + 1)],
                             wm[:, WV_OFF + 64 * cc:WV_OFF + 64 * (cc + 1)],
                             start=(cc == 0), stop=(cc == NCC - 1)), f"mm_v{j}")
    if CPV_ENG == "scalar":
        L(nc.scalar.copy(vaug[:, 4 * j:4 * j + 4, 0:H], pv[:, 0:256]), f"cp_v{j}")
    else:
        L(nc.vector.tensor_copy(vaug[:, 4 * j:4 * j + 4, 0:H], pv[:, 0:256]),
          f"cp_v{j}")


def emit_attention(cx, ph, next_ph=None, self_proj=False):
    """One full attention pass over proj set `ph`.  If next_ph is given,
    interleave the next iteration's xt DMA + projections as low-priority
    filler.  If self_proj (single-shot path), blocks 1..3 of THIS phase's
    projections are hoisted into the stream at high priority (the caller
    emits only block 0's projections up front)."""
    nc, tc = cx.nc, cx.tc
    proj_a, proj_b, vaug, _ = cx.projs[ph]
    wm = cx.wm
    tri = wm[:, TRI_OFF:TRI_OFF + 128]
    Exp = mybir.ActivationFunctionType.Exp
    po_tiles = {}

    def make_pair(j, i0, i1, pv_flags, use_dve):
        d0, d1 = i0 - 4 * j, i1 - 4 * j
        c0 = 128 * max(0, d0)
        c1 = 128 * max(0, d1)
        s1 = 512 - c1
        state = {}

        def qk_exp_mask():
            wp = cx.psw.tile([128, 1024], F32, tag="w")
            L(nc.tensor.matmul(wp[:, c0:512],
                             proj_b[0:64, 128 * i0:128 * (i0 + 1)],
                             proj_a[0:64, 512 * j + c0:512 * (j + 1)],
                             start=True, stop=True), f"QK_j{j}i{i0}")
            L(nc.tensor.matmul(wp[:, 512:512 + s1],
                             proj_a[64:128, 128 * i1:128 * (i1 + 1)],
                             proj_b[64:128, 512 * j + c1:512 * (j + 1)],
                             start=True, stop=True, tile_position=(64, 0)),
              f"QK_j{j}i{i1}")
            if use_dve:
                e = cx.epool.tile([128, 1024], U16, tag="e", name=f"e{ph}_{j}_{i0}")
                L(nc.vector.tensor_scalar(
                    e[:, c0:512 + s1], wp[:, c0:512 + s1], EXP_A, EXP_B,
                    mybir.AluOpType.mult, mybir.AluOpType.add), f"dexp_j{j}i{i0}")
            else:
                e = cx.epool.tile([128, 1024], BF, tag="e", name=f"e{ph}_{j}_{i0}")
                L(nc.scalar.activation(e[:, c0:512 + s1], wp[:, c0:512 + s1], Exp,
                                     scale=0.125), f"exp_j{j}i{i0}")
            state["e"] = e

        def pv():
            e = state.pop("e")
            if d0 >= 0:
                meng = nc.vector if MASK_ENG == "vector" else nc.gpsimd
                L(meng.tensor_mul(e[:, c0:c0 + 128], e[:, c0:c0 + 128], tri),
                  f"mask_j{j}i{i0}")
                L(meng.tensor_mul(e[:, 512:640], e[:, 512:640], tri),
                  f"mask_j{j}i{i1}")
            po = po_tiles[j]
            # transposed PV: strip 0 sits at its natural block offset
            # (group g at e col 128g); strip 1 packed at [512 : 512+s1]
            for chunk_sel, g, st, sp in pv_flags:
                if chunk_sel == 0:
                    i, col = i0, 128 * g
                else:
                    i, col = i1, 512 + 128 * (g - max(0, d1))
                es = e[:, col:col + 128]
                if use_dve:
                    es = es.bitcast(BF)
                L(nc.tensor.matmul(po[:, g, 0:H + 1], es, vaug[:, i, 0:H + 1],
                                 start=st, stop=sp, skip_group_check=True),
                  f"PV_j{j}i{i}g{g}")

        return qk_exp_mask, pv

    def emit_epi(j):
        po = po_tiles.pop(j)
        rc = cx.onat.tile([128, 4, 1], F32, tag="rc")
        L(nc.vector.reciprocal(rc[:], po[:, :, H:H + 1]), f"rc{j}")
        on = cx.onat.tile([128, 4, H], F32, tag="on")
        L(nc.vector.tensor_mul(on[:], po[:, :, 0:H],
                             rc[:].to_broadcast([128, 4, H])), f"on{j}")
        # (emission order already places next-iteration xt DMAs ahead of
        # these in the SP queue, so they don't gate the pipeline)
        L(nc.sync.dma_start(cx.out_d[:, slice(4 * j, 4 * j + 4), :], on[:]),
          f"dma_out{j}")

    # ---- build the unit stream ------------------------------------------
    stream = []
    for j in range(NTQ):
        order = [(4 * j, 4 * j + 3), (4 * j + 1, 4 * j + 2)]
        order += [(2 * p, 2 * p + 1) for p in range(2 * j)]
        seq = []
        for idx, (i0, i1) in enumerate(order):
            d0, d1 = i0 - 4 * j, i1 - 4 * j
            if j == NTQ - 1 and idx == len(order) - 1:
                for g in range(4):
                    if g >= max(0, d0):
                        seq.append((idx, 0, g))
                    if g >= max(0, d1):
                        seq.append((idx, 1, g))
            else:
                for g in range(max(0, d0), 4):
                    seq.append((idx, 0, g))
                for g in range(max(0, d1), 4):
                    seq.append((idx, 1, g))
        flags_by_unit = {idx: [] for idx in range(len(order))}
        for pos, (idx, cs, g) in enumerate(seq):
            flags_by_unit[idx].append((cs, g, pos == 0, pos == len(seq) - 1))
        for idx, (i0, i1) in enumerate(order):
            qk, pv = make_pair(j, i0, i1, flags_by_unit[idx],
                               use_dve=(idx in _dve_units(j, len(order))))
            stream.append((j, idx, qk, pv))

    # ---- next-iteration proj filler pieces ------------------------------
    filler = []
    if next_ph is not None:
        filler = [lambda: emit_xt_dma(cx, next_ph)]
        filler += [(lambda jj: lambda: emit_proj(cx, next_ph, jj))(j)
                   for j in range(NTQ)]
        filler += [(lambda jj: lambda: emit_vproj(cx, next_ph, jj))(j)
                   for j in range(NTQ)]
    # spread the 9 filler pieces across the first 18 units
    filler_at = {2 * i + 1: f for i, f in enumerate(filler)}

    hp = tc.high_priority
    lp = lambda off: tc.high_priority(offset=-off)

    pv_pending = []
    for u, (j, idx, qk, pv) in enumerate(stream):
        if idx == 0:
            po_tiles[j] = cx.pso.tile([128, 4, H + 1], F32, tag="o",
                                      name=f"po{ph}_{j}")
        with hp():
            qk()
        if self_proj:
            if j == 0 and idx == 0:
                with hp():
                    emit_proj(cx, ph, 1)
            if j == 0 and idx == 1:
                with hp():
                    emit_vproj(cx, ph, 0)
                    emit_vproj(cx, ph, 1)
                    emit_proj(cx, ph, 2)
            if j == 1 and idx == 1:
                with hp():
                    emit_proj(cx, ph, 3)
                    emit_vproj(cx, ph, 2)
            if j == 2 and idx == 1:
                with hp():
                    emit_vproj(cx, ph, 3)
        if pv_pending:
            with lp(10000):
                pv_pending.pop(0)()
        if idx == 0 and j >= 1:
            with lp(12000):
                emit_epi(j - 1)
        if u in filler_at:
            with lp(20000):
                filler_at[u]()
        pv_pending.append(pv)
    for pv in pv_pending:
        with hp():
            pv()
    with hp():
        emit_epi(NTQ - 1)


def build_nc(repeats=1):
    """Build the per-core Bass program (SPMD: same program on all 8 cores).

    repeats > 1 wraps two software-pipelined attention phases in an
    on-device For_i loop (repeats must be even); used by the benchmarking
    harness to amortize host/launch overhead out of timing.
    """
    _patch_tile_drain()
    nc = bass.Bass("TRN2", target_bir_lowering=False, debug=False)

    xt_d = nc.dram_tensor("xt", [128, NCC, T], BF, kind="ExternalInput")
    wm_d = nc.dram_tensor("wm", [128, WMISC_COLS], BF, kind="ExternalInput")
    out_d = nc.dram_tensor("out", [128, NTK, H], F32, kind="ExternalOutput")
    dram = (xt_d, wm_d, out_d)

    with tile.TileContext(nc) as tc:
        with (
            tc.tile_pool(name="persist", bufs=1) as persist,
            tc.tile_pool(name="xtp", bufs=1) as xtp,
            tc.tile_pool(name="epool", bufs=14) as epool,
            tc.tile_pool(name="onat", bufs=4) as onat,
            tc.tile_pool(name="psw", bufs=PSW_BUFS, space="PSUM") as psw,
            tc.tile_pool(name="pso", bufs=PSO_BUFS, space="PSUM") as pso,
            tc.tile_pool(name="pmisc", bufs=PMISC_BUFS, space="PSUM") as pmisc,
        ):
            wm = persist.tile([128, WMISC_COLS], BF, tag="wm")
            n_ph = 1 if repeats == 1 else 2
            projs = []
            for ph in range(n_ph):
                proj_a = persist.tile([128, T], BF, tag=f"proj_a{ph}")
                proj_b = persist.tile([128, T], BF, tag=f"proj_b{ph}")
                vaug = persist.tile([128, NTK, H + 2], BF, tag=f"vaug{ph}")
                xt = xtp.tile([128, NCC, T], BF, tag=f"xt{ph}")
                projs.append((proj_a, proj_b, vaug, xt))

            L(nc.sync.dma_start(wm[:, :], wm_d[:, :]), "dma_wm")
            for ph in range(n_ph):
                nc.gpsimd.memset(projs[ph][2][:, :, H:H + 1], 1.0)
            warm = onat.tile([1, 16], F32, tag="warm")
            nc.vector.memset(warm[:], 0.0)
            nc.scalar.activation(warm[:], warm[:],
                                 mybir.ActivationFunctionType.Exp, scale=1.0)

            pools = (xtp, epool, onat, psw, pso, pmisc)
            cx = Ctx(nc, tc, pools, wm, projs, dram)

            emit_xt_dma(cx, 0)
            if repeats == 1:
                # single-shot: only block 0's projections up front, the rest
                # hoisted into the attention stream
                emit_proj(cx, 0, 0)
                emit_attention(cx, 0, next_ph=None, self_proj=True)
            else:
                # prologue: all projections for the first pass
                for j in range(NTQ):
                    emit_proj(cx, 0, j)
                    emit_vproj(cx, 0, j)
                assert repeats % 2 == 0, "pipelined loop needs even repeats"
                with tc.For_i(0, repeats // 2, 1):
                    emit_attention(cx, 0, next_ph=1)
                    emit_attention(cx, 1, next_ph=0)
    _split_excess_waits(nc)
    return nc


def make_in_maps(x, Wk, Wq, Wv):
    """Host-side layout prep: per-core transposed x, packed weights, masks."""
    import ml_dtypes
    bf16 = ml_dtypes.bfloat16

    x = np.asarray(x, dtype=np.float32)
    Wk = np.asarray(Wk, dtype=np.float32)
    Wq = np.asarray(Wq, dtype=np.float32)
    Wv = np.asarray(Wv, dtype=np.float32)

    wa = np.concatenate([Wq, Wk], axis=1).reshape(NCC, 128, 128)
    wv = Wv.reshape(NCC, 128, H)
    r = np.arange(128)
    tri = (r[:, None] <= r[None, :]).astype(np.float32)  # keep tk <= tq
    perm = np.zeros((128, 128), dtype=np.float32)
    perm[r, (r + 64) % 128] = 1.0
    wm = np.concatenate(
        [wa[0], wa[1], wv[0], wv[1], tri, perm], axis=1).astype(bf16)
    assert wm.shape == (128, WMISC_COLS)

    common = {"wm": np.ascontiguousarray(wm)}
    in_maps = []
    for b in range(B):
        xt = np.ascontiguousarray(
            np.transpose(x[b].T.reshape(NCC, 128, T), (1, 0, 2))).astype(bf16)
        in_maps.append({"xt": xt, **common})
    return in_maps


def unpack_out(raw):
    """Device layout [128, NTK, H] (partition-major) -> [T, H]."""
    return np.ascontiguousarray(
        np.transpose(raw, (1, 0, 2)).reshape(T, H))


def kernel(x, Wk, Wq, Wv):
    nc = build_nc(repeats=1)
    in_maps = make_in_maps(x, Wk, Wq, Wv)
    res = bass_utils.run_bass_kernel_spmd(nc, in_maps, core_ids=list(range(B)))
    return np.stack([unpack_out(res.results[b]["out"]) for b in range(B)],
                    axis=0)
